# revision 4
# baseline (speedup 1.0000x reference)
"""Trainium2 Bass kernel for nn_BiSpikeNet — v2 sign-domain rebalance.

Work in V = 2*Yhat = 2*m/c. Host pre-scales the input: xin_0 = 2*invc*x_0,
xin_t = 2*invc*x_t - 1 (t>=1). Spikes are stored as G = sign(V - 2*theta)
in {-1,+1} (bf16), so S = (G+1)/2:
  V_t = xin_t + W_{t-1}                 (tt add, split GPSIMD/DVE)
  rowabs = sum_row|V|                   (ACT Abs accum [0:CA] + DVE
                                         tensor_reduce(abs) [CA:2048])
  psTG[P,1] = 2*theta = k.allK^T@rowabs + eps   (PE, bcast for free)
  ghat2 = 1/psTG ; ghat = 2*ghat2       (DVE tiny)
  G_t = Sign(V*ghat2 - 1) -> bf16       (ACT, accum -> sign-sums)
  W_t = ghat*V - G_t                    (DVE stt halves; dead at t=T-1)
Counts fixup (S-sums from G-sums) is folded into the host-side MLP
weights: w1' = W1/(2F), b1' = b1 + 0.5*sum_t W1. Phase 2 computes
out = 0.5 + sum_t (0.5*aw_t)*G_t: identity is host-scaled by 0.5 and the
PSUM->SBUF copies add bias 0.5.
"""

import os
import numpy as np
import ml_dtypes

P = 128
FREE = 2048
HF = 1024
T = 8
BL = 2
NCORES = 8
NSLAB = T * BL
F = 256 * 32 * 32
NH, HID = 4, 64

# column splits: gps add [0:GA], DVE add [GA:FREE]; ACT abs [0:GA], DVE reduce [GA:]
GA = 1300

_cache = {}
LAST_RESULT = None


def _build(vth, invc):
    import concourse.bacc as bacc
    import concourse.mybir as mybir
    import concourse.tile as tile

    dt = mybir.dt
    Alu = mybir.AluOpType
    Act = mybir.ActivationFunctionType

    nc = bacc.Bacc("TRN2", target_bir_lowering=False, debug=False,
                   num_devices=NCORES)

    x_d = nc.declare_dram_parameter("x", [NSLAB, P, FREE], dt.float32, isOutput=False)
    w1_d = nc.declare_dram_parameter("w1", [2, P, T], dt.float32, isOutput=False)
    b1_d = nc.declare_dram_parameter("b1", [2, P, 1], dt.float32, isOutput=False)
    w2_d = nc.declare_dram_parameter("w2", [2, P, T], dt.float32, isOutput=False)
    b2_d = nc.declare_dram_parameter("b2", [2, 2, 2 * T], dt.float32, isOutput=False)
    aw_d = nc.declare_dram_parameter("attw", [2, 2, 1], dt.float32, isOutput=False)
    gones_d = nc.declare_dram_parameter("gones", [P, 2], dt.float32, isOutput=False)
    identb_d = nc.declare_dram_parameter("identb", [P, P], dt.bfloat16, isOutput=False)
    out_d = nc.declare_dram_parameter("out", [BL, P, FREE], dt.float32, isOutput=True)

    k_theta = float(np.float32(vth) / np.float32(2 * F))
    # eps row value: k_theta * 128 * e = 2e-6 * vth * invc
    eps_val = float(1e-6 * float(invc) * F / 128.0)

    with tile.TileContext(nc) as tc:
        with (
            tc.tile_pool(name="xp", bufs=4) as xp,
            tc.tile_pool(name="yp", bufs=3) as yp,
            tc.tile_pool(name="scrp", bufs=2) as scrp,
            tc.tile_pool(name="persist", bufs=1) as pp,
            tc.tile_pool(name="small", bufs=4) as sp,
            tc.tile_pool(name="posbp", bufs=6) as pb,
            tc.tile_pool(name="psmall", bufs=1, space="PSUM") as psm,
            tc.tile_pool(name="psout", bufs=6, space="PSUM") as pso,
        ):
            # ---- first x slabs: dedicated tiles, DMA'd before everything ----
            x0 = pp.tile([P, FREE], dt.float32, tag="x0")
            nc.sync.dma_start(x0[:], x_d[0, :, :])
            x1 = pp.tile([P, FREE], dt.float32, tag="x1")
            nc.sync.dma_start(x1[:], x_d[1, :, :])

            # ---- persistent aux ----
            allones = pp.tile([P, P], dt.float32, tag="allones")
            nc.vector.memset(allones[:], 1.0)
            ones_row = pp.tile([1, P], dt.float32, tag="ones_row")
            nc.vector.memset(ones_row[:], 1.0)
            ones2 = pp.tile([2, 1], dt.float32, tag="ones2")
            nc.vector.memset(ones2[:], 1.0)
            eps128 = pp.tile([P, 1], dt.float32, tag="eps128")
            nc.vector.memset(eps128[:], eps_val)
            allK = pp.tile([P, P], dt.float32, tag="allK")
            nc.vector.memset(allK[:], k_theta)
            neg1 = pp.tile([P, 1], dt.float32, tag="neg1")
            nc.vector.memset(neg1[:], -2.0)
            identb = pp.tile([P, P], dt.bfloat16, tag="identb")
            nc.sync.dma_start(identb[:], identb_d[:, :])
            gones = pp.tile([P, 2], dt.float32, tag="gones")
            nc.sync.dma_start(gones[:], gones_d[:, :])
            w1sb, b1sb, w2sb, b2sb, awsb = [], [], [], [], []
            for l in range(2):
                w1t = pp.tile([P, T], dt.float32, tag=f"w1_{l}")
                nc.sync.dma_start(w1t[:], w1_d[l, :, :])
                w1sb.append(w1t)
                b1t = pp.tile([P, 1], dt.float32, tag=f"b1_{l}")
                nc.sync.dma_start(b1t[:], b1_d[l, :, :])
                b1sb.append(b1t)
                w2t = pp.tile([P, T], dt.float32, tag=f"w2_{l}")
                nc.sync.dma_start(w2t[:], w2_d[l, :, :])
                w2sb.append(w2t)
                b2t = pp.tile([2, 2 * T], dt.float32, tag=f"b2_{l}")
                nc.sync.dma_start(b2t[:], b2_d[l, :, :])
                b2sb.append(b2t)
                awt = pp.tile([2, 1], dt.float32, tag=f"aw_{l}")
                nc.sync.dma_start(awt[:], aw_d[l, :, :])
                awsb.append(awt)

            wstate = [pp.tile([P, FREE], dt.float32, tag=f"wst{b}", name=f"wst{b}")
                      for b in range(BL)]
            spikes = [pp.tile([P, FREE], dt.bfloat16, tag=f"s{i}", name=f"s{i}")
                      for i in range(NSLAB)]
            rowcnts = pp.tile([P, 2 * NSLAB], dt.float32, tag="rowcnts")

            # ---- phase 1: T-step recursion ----
            x_tiles = {}

            def emit_x(i):
                if i >= NSLAB:
                    return
                if i == 0:
                    x_tiles[0] = x0
                elif i == 1:
                    x_tiles[1] = x1
                else:
                    xt = xp.tile([P, FREE], dt.float32, tag="xt", name=f"xt{i}")
                    nc.sync.dma_start(xt[:, 0:HF], x_d[i, :, 0:HF])
                    nc.sync.dma_start(xt[:, HF:FREE], x_d[i, :, HF:FREE])
                    x_tiles[i] = xt

            for i in range(4):
                emit_x(i)

            def emit_back(prev):
                """sign + wstate of the previous slab (ACT first, then DVE)."""
                if prev is None:
                    return
                pi, pb_, pyt, pghat = prev
                for h in range(2):
                    lo = h * HF
                    nc.scalar.activation(
                        spikes[pi][:, lo:lo + HF], pyt[:, lo:lo + HF],
                        Act.Sign, bias=neg1[:, 0:1], scale=pghat[:, 0:1],
                        accum_out=rowcnts[:, 2 * pi + h:2 * pi + h + 1])
                if pi // BL < T - 1:
                    for h in range(2):
                        lo = h * HF
                        nc.vector.scalar_tensor_tensor(
                            wstate[pb_][:, lo:lo + HF], pyt[:, lo:lo + HF],
                            pghat[:, 0:1], spikes[pi][:, lo:lo + HF],
                            Alu.mult, Alu.subtract)

            prev = None
            for t in range(T):
                for b in range(BL):
                    i = t * BL + b
                    xt = x_tiles.pop(i)
                    emit_x(i + 4)
                    # front(i): add + abs + theta + ghat. gps add is emitted
                    # before back(i-1) so it runs concurrently with it.
                    if t == 0:
                        yt = xt
                    else:
                        yt = yp.tile([P, FREE], dt.float32, tag="yt",
                                     name=f"y{i}")
                        nc.gpsimd.tensor_tensor(
                            yt[:, 0:GA], xt[:, 0:GA],
                            wstate[b][:, 0:GA], Alu.add)
                        nc.vector.tensor_tensor(
                            yt[:, GA:FREE], xt[:, GA:FREE],
                            wstate[b][:, GA:FREE], Alu.add)
                    rowabs = sp.tile([P, 2], dt.float32, tag="rowabs",
                                     name=f"ra{i}")
                    nc.vector.tensor_reduce(
                        rowabs[:, 1:2], yt[:, GA:FREE], mybir.AxisListType.X,
                        Alu.add, apply_absolute_value=True)
                    # back(i-1): signs (ACT) + wstate (DVE) run while gps
                    # grinds through add(i)
                    emit_back(prev)
                    # abs(i) on ACT after the signs
                    scr = scrp.tile([P, GA], dt.bfloat16, tag="scr")
                    nc.scalar.activation(
                        scr[:], yt[:, 0:GA], Act.Abs,
                        bias=0.0, scale=1.0, accum_out=rowabs[:, 0:1])
                    psTG = psm.tile([P, 1], dt.float32,
                                    tag=("psA" if b == 0 else "psB"),
                                    name=f"psTG{i}")
                    nc.tensor.matmul(psTG[:], allK[:], rowabs[:, 1:2],
                                     start=True, stop=False)
                    nc.tensor.matmul(psTG[:], allK[:], rowabs[:, 0:1],
                                     start=False, stop=True)
                    ghat = sp.tile([P, 1], dt.float32, tag="ghat",
                                   name=f"gh{i}")
                    nc.vector.reciprocal(ghat[:, 0:1], psTG[:])
                    prev = (i, b, yt, ghat)
            emit_back(prev)

            # ---- counts -> all-partition totals ----
            psN = psm.tile([P, 2 * NSLAB], dt.float32, tag="psB", name="psN")
            nc.tensor.matmul(psN[:], allones[:], rowcnts[:], start=True, stop=True)
            cntsb = pp.tile([P, 2 * NSLAB], dt.float32, tag="cntsb")
            nc.vector.tensor_copy(cntsb[:], psN[:])
            cnt = pp.tile([P, NSLAB], dt.float32, tag="cnt")
            # combine the two halves of each slab
            nc.vector.tensor_tensor(cnt[:], cntsb[:, 0:2 * NSLAB:2],
                                    cntsb[:, 1:2 * NSLAB:2], Alu.add)

            # ---- MLP attention + softmax, both samples batched ----
            # krow16 layout: cols [0:T] = sample 0, [T:2T] = sample 1
            krow = pp.tile([1, NSLAB], dt.float32, tag="krow")
            kI = {}
            kbc = [pp.tile([P, T], dt.float32, tag=f"kbc{b}", name=f"kbc{b}")
                   for b in range(BL)]
            mws = []
            for l in range(2):
                mc = sp.tile([P, 2 * T], dt.float32, tag=f"mc{l}", name=f"mc{l}")
                for b in range(BL):
                    junk = sp.tile([P, T], dt.float32, tag=f"junk{l}{b}",
                                   name=f"junk{l}{b}")
                    hraw = sp.tile([P, 1], dt.float32, tag=f"hraw{l}{b}",
                                   name=f"hraw{l}{b}")
                    nc.vector.scalar_tensor_tensor(
                        junk[:], w1sb[l][:], 1.0, cnt[:, b::BL],
                        Alu.mult, Alu.mult, accum_out=hraw[:])
                    hcol = sp.tile([P, 1], dt.float32, tag=f"hcol{l}{b}",
                                   name=f"hcol{l}{b}")
                    nc.scalar.activation(hcol[:], hraw[:], Act.Relu,
                                         bias=b1sb[l][:, 0:1], scale=1.0)
                    nc.vector.tensor_scalar(mc[:, b * T:(b + 1) * T],
                                            w2sb[l][:], hcol[:, 0:1], None,
                                            Alu.mult)
                psM = psm.tile([2, 2 * T], dt.float32, tag="psA", name=f"psM{l}")
                nc.tensor.matmul(psM[:], gones[:], mc[:], start=True, stop=True)
                mp = sp.tile([2, 2 * T], dt.float32, tag=f"mp{l}", name=f"mp{l}")
                nc.vector.tensor_tensor(mp[:], psM[:], b2sb[l][:], Alu.add)
                mw = sp.tile([2, 2 * T], dt.float32, tag=f"mw{l}", name=f"mw{l}")
                nc.vector.tensor_scalar(mw[:], mp[:], awsb[l][:, 0:1], None,
                                        Alu.mult)
                mws.append(mw)
            psW = psm.tile([1, 2 * T], dt.float32, tag="psA", name="psW")
            nc.tensor.matmul(psW[:], ones2[:], mws[0][:], start=True, stop=False)
            nc.tensor.matmul(psW[:], ones2[:], mws[1][:], start=False, stop=True)
            wt = sp.tile([1, 2 * T], dt.float32, tag="wt")
            nc.vector.tensor_copy(wt[:], psW[:])
            for b in range(BL):
                sl = slice(b * T, (b + 1) * T)
                mx = sp.tile([1, 1], dt.float32, tag=f"mx{b}", name=f"mx{b}")
                nc.vector.tensor_reduce(mx[:], wt[0:1, sl], mybir.AxisListType.X,
                                        Alu.max)
                nmx = sp.tile([1, 1], dt.float32, tag=f"nmx{b}", name=f"nmx{b}")
                nc.vector.tensor_scalar(nmx[:], mx[:], -1.0, None, Alu.mult)
                ex = sp.tile([1, T], dt.float32, tag=f"ex{b}", name=f"ex{b}")
                nc.scalar.activation(ex[:], wt[0:1, sl], Act.Exp,
                                     bias=nmx[0:1, 0:1], scale=1.0)
                zs = sp.tile([1, 1], dt.float32, tag=f"zs{b}", name=f"zs{b}")
                nc.vector.tensor_reduce(zs[:], ex[:], mybir.AxisListType.X,
                                        Alu.add)
                rz = sp.tile([1, 1], dt.float32, tag=f"rz{b}", name=f"rz{b}")
                nc.vector.reciprocal(rz[:], zs[:])
                nc.vector.tensor_scalar(krow[0:1, sl], ex[:], rz[0:1, 0:1],
                                        None, Alu.mult)
                # broadcast this sample's 8 weights to all partitions
                psK = psm.tile([P, T], dt.float32, tag="psB", name=f"psK{b}")
                nc.tensor.matmul(psK[:], ones_row[:], krow[0:1, sl],
                                 start=True, stop=True)
                nc.vector.tensor_copy(kbc[b][:], psK[:])
                # kI for this sample immediately (overlaps the other sample's
                # softmax on ACT/PE)
                for t_ in range(T):
                    i_ = t_ * BL + b
                    kt = pp.tile([P, P], dt.bfloat16, tag=f"ki{i_}",
                                 name=f"ki{i_}")
                    nc.vector.tensor_scalar(kt[:], identb[:],
                                            kbc[b][:, t_:t_ + 1], None,
                                            Alu.mult)
                    kI[i_] = kt



            # ---- phase 2: out[b] = sum_t k * S, t-outer PSUM waves ----
            NCH = FREE // 512
            waves = [[(0, 0), (0, 1), (0, 2), (0, 3), (1, 0), (1, 1)],
                     [(1, 2), (1, 3)]]
            for wave in waves:
                po = {}
                for (b, ch) in wave:
                    po[(b, ch)] = pso.tile([P, 512], dt.float32, tag="po",
                                           name=f"po{b}_{ch}")
                for t in range(T):
                    for (b, ch) in wave:
                        i = t * BL + b
                        nc.tensor.matmul(po[(b, ch)][:], kI[i][:],
                                         spikes[i][:, ch * 512:(ch + 1) * 512],
                                         start=(t == 0), stop=(t == T - 1))
                for j, (b, ch) in enumerate(wave):
                    posb = pb.tile([P, 512], dt.float32, tag="posb")
                    if j % 2 == 0:
                        nc.scalar.activation(posb[:], po[(b, ch)][:],
                                             Act.Copy, bias=0.5, scale=1.0)
                    else:
                        nc.vector.tensor_scalar(posb[:], po[(b, ch)][:],
                                                0.5, None, Alu.add)
                    nc.sync.dma_start(out_d[b, :, ch * 512:(ch + 1) * 512],
                                      posb[:])

    nc.compile()
    return nc


def kernel(**inputs):
    global LAST_RESULT
    from concourse.bass_utils import run_bass_kernel_spmd

    x = np.ascontiguousarray(np.asarray(inputs["x"], dtype=np.float32))
    decay_param = np.float32(np.asarray(inputs["decay_param"], dtype=np.float32))
    v_th = np.float32(np.asarray(inputs["v_th"], dtype=np.float32))
    W1 = np.asarray(inputs["W1"], dtype=np.float32)
    b1 = np.asarray(inputs["b1"], dtype=np.float32)
    W2 = np.asarray(inputs["W2"], dtype=np.float32)
    b2 = np.asarray(inputs["b2"], dtype=np.float32)
    att_w = np.asarray(inputs["att_w"], dtype=np.float32)

    Tn, B, C, H, W = x.shape
    assert (Tn, B, C * H * W) == (T, BL * NCORES, F)

    d = np.float32(1.0) / (np.float32(1.0) + np.float32(np.exp(-np.float64(decay_param))))
    c = np.float32(d * v_th)
    invc = np.float32(1.0) / c

    key = (float(v_th), float(invc))
    nc = _cache.get(key)
    if nc is None:
        nc = _build(float(v_th), float(invc))
        _cache[key] = nc

    w1c = (W1 / np.float32(2 * F)).reshape(NH * HID, T).reshape(2, P, T)
    b1c = (b1 + np.float32(0.5) * W1.sum(axis=2)).reshape(NH * HID).reshape(2, P, 1)
    w2c = W2.transpose(0, 2, 1).reshape(NH * HID, T).reshape(2, P, T)
    b2c = np.tile(b2.reshape(2, 2, T), (1, 1, 2))
    awc = att_w.reshape(2, 2, 1)
    gones = np.zeros((P, 2), dtype=np.float32)
    gones[0:64, 0] = 1.0
    gones[64:128, 1] = 1.0
    identb = (np.float32(0.5) * np.eye(P, dtype=np.float32)).astype(ml_dtypes.bfloat16)

    aux = {"w1": np.ascontiguousarray(w1c, np.float32),
           "b1": np.ascontiguousarray(b1c, np.float32),
           "w2": np.ascontiguousarray(w2c, np.float32),
           "b2": np.ascontiguousarray(b2c, np.float32),
           "attw": np.ascontiguousarray(awc, np.float32),
           "gones": gones, "identb": identb}

    in_maps = []
    two_invc = np.float32(2.0) * invc
    for m in range(NCORES):
        xm = (x[:, m * BL:(m + 1) * BL].reshape(NSLAB, P, FREE) * two_invc)
        xm[BL:] -= np.float32(1.0)
        im = {"x": np.ascontiguousarray(xm)}
        im.update(aux)
        in_maps.append(im)

    trace = os.environ.get("BISPIKE_PROFILE", "") == "1"
    res = run_bass_kernel_spmd(nc, in_maps, list(range(NCORES)), trace=trace)
    LAST_RESULT = res

    out = np.empty((B, F), dtype=np.float32)
    for m in range(NCORES):
        out[m * BL:(m + 1) * BL] = res.results[m]["out"].reshape(BL, F)
    return out
